# revision 9
# baseline (speedup 1.0000x reference)
"""HEX loss kernel for Trainium2 (8 NeuronCores, batch-parallel, raw Bass).

Math: the chain junction-tree potential is rank-1 per clique and each
interior fs[v] is split fs[v]/2 over its two cliques, so the joint
distribution factorizes into independent Bernoullis with
P(y_v=1) = sigmoid(fs[b,v]); hence
    loss = mean_b softplus(-fs[b, labels[b]])

Implementation (per core, 4096 rows): dma_gather pulls ONE 256-byte
chunk (64 f32) per row -- the chunk containing fs[b, lab_b] -- via
host-precomputed int16 indices (4*b + (lab>>6)); 4 gathers x 1024 idxs
on SWDGE queues 0-3 (1 MB HBM traffic), preceded by a 128-idx warmup
gather to absorb Q7 descriptor-gen cold-start. Select + softplus are
fully fused: DVE builds a one-hot mask via is_equal(iota64, lab&63)
with broadcast APs, then one scalar_tensor_tensor per quarter forms
prod = (fs - 20)*mask (selected -> fs-20, else 0). ACT computes
u = Exp(-prod - 20) (selected -> e^-fs, else e^-20 ~ 2e-9) and
Ln(1 + u) with accum_out, so every element's softplus lands in a
per-partition accumulator with no DVE reduce at all. A PE matmul with
a ones vector reduces [128,1] -> [1,1] in PSUM so the final output DMA
is a single descriptor (a [128,1] DRAM store costs ~7-10 us in
16-engine completion-sem stagger). Host sums 8 scalars / B.
"""

import numpy as np

B = 32768
V = 256
N_CORES = 8
BL = B // N_CORES          # 4096 rows per core
P = 128
CHUNK = 64                 # f32 elems per gathered chunk (256 B, SWDGE minimum)
CPR = V // CHUNK           # 4 chunks per row
NQ = 4                     # gather instructions (SWDGE queues)
IPG = BL // NQ             # 1024 idxs per gather
JPG = IPG // P             # 8 dst columns per gather
NJ = BL // P               # 32 chunk columns total
NLE_TABLE_ID = 6           # natural_log_exp_and_others in gen3 act_info
PEN = 20.0                 # penalty offset: exp(-PEN) ~ 2e-9 for masked-out elems

_CACHE = {}


def _build():
    from contextlib import ExitStack

    import concourse.bass as bass  # noqa
    from concourse import bacc, mybir
    from concourse.library_config import mlp

    f32 = mybir.dt.float32
    i16 = mybir.dt.int16
    Alu = mybir.AluOpType
    Act = mybir.ActivationFunctionType

    nc = bacc.Bacc(
        "TRN2",
        target_bir_lowering=False,
        debug=False,
        enable_asserts=True,
        num_devices=N_CORES,
        num_swdge_queues=NQ,
    )

    fs_d = nc.dram_tensor("fs", [BL, V], f32, kind="ExternalInput").ap()
    idx_d = nc.dram_tensor("idx", [P, BL // 16], i16, kind="ExternalInput").ap()
    cst_d = nc.dram_tensor("consts", [P, NJ + CHUNK + 2], f32, kind="ExternalInput").ap()
    out_d = nc.dram_tensor("out", [1, 1], f32, kind="ExternalOutput").ap()

    fs_rows = fs_d.rearrange("b (c v) -> (b c) v", c=CPR)  # [16384, 64]

    with ExitStack() as ctx:
        idx_sb = ctx.enter_context(nc.sbuf_tensor("idx_sb", [P, BL // 16], i16))
        cst = ctx.enter_context(nc.sbuf_tensor("cst", [P, NJ + CHUNK + 2], f32))
        dst = ctx.enter_context(nc.sbuf_tensor("dst", [P, NJ * CHUNK], f32))
        warm = ctx.enter_context(nc.sbuf_tensor("warm", [P, CHUNK], f32))
        mask = ctx.enter_context(nc.sbuf_tensor("mask", [P, NJ * CHUNK], f32))
        prod = ctx.enter_context(nc.sbuf_tensor("prod", [P, NJ * CHUNK], f32))
        u_big = ctx.enter_context(nc.sbuf_tensor("u_big", [P, NJ * CHUNK], f32))
        ln_big = ctx.enter_context(nc.sbuf_tensor("ln_big", [P, NJ * CHUNK], f32))
        acc4 = ctx.enter_context(nc.sbuf_tensor("acc4", [P, NQ], f32))
        junk4 = ctx.enter_context(nc.sbuf_tensor("junk4", [P, NQ], f32))
        acc1 = ctx.enter_context(nc.sbuf_tensor("acc1", [P, 1], f32))
        outs = ctx.enter_context(nc.sbuf_tensor("outs", [1, 1], f32))
        po = ctx.enter_context(nc.psum_tensor("po", [1, 1], f32))

        s_i = ctx.enter_context(nc.semaphore("s_i"))
        s_c = ctx.enter_context(nc.semaphore("s_c"))
        s_w = ctx.enter_context(nc.semaphore("s_w"))
        s_g = [ctx.enter_context(nc.semaphore(f"s_g{q}")) for q in range(NQ)]
        s_sel = ctx.enter_context(nc.semaphore("s_sel"))
        s_acc = ctx.enter_context(nc.semaphore("s_acc"))
        s_mm = ctx.enter_context(nc.semaphore("s_mm"))
        s_out = ctx.enter_context(nc.semaphore("s_out"))

        labm_ap = cst.ap()[:, 0:NJ]                        # [128, 32]
        iota_ap = cst.ap()[:, NJ : NJ + CHUNK]             # [128, 64]
        ones_ap = cst.ap()[:, NJ + CHUNK : NJ + CHUNK + 1]  # [128, 1]
        npen_ap = cst.ap()[:, NJ + CHUNK + 1 : NJ + CHUNK + 2]  # [128, 1] of -PEN

        blk = ctx.enter_context(nc.Block())

        @blk.sync
        def _(s_eng):
            s_eng.dma_start(out=idx_sb.ap(), in_=idx_d).then_inc(s_i, 16)
            s_eng.dma_start(out=cst.ap(), in_=cst_d).then_inc(s_c, 16)

        @blk.gpsimd
        def _(g_eng):
            g_eng.load_library(mlp)
            g_eng.wait_ge(s_i, 16)
            # warmup: absorb Q7 desc-gen cold-start before the real gathers
            g_eng.dma_gather(
                warm.ap().rearrange("p (j v) -> p j v", j=1),
                fs_rows,
                idx_sb.ap()[:, 0:8],
                P,
                P,
                CHUNK,
                queue_num=0,
            ).then_inc(s_w, 16)
            for q in range(NQ):
                g_eng.dma_gather(
                    dst.ap()[:, q * JPG * CHUNK : (q + 1) * JPG * CHUNK].rearrange(
                        "p (j v) -> p j v", j=JPG
                    ),
                    fs_rows,
                    idx_sb.ap()[:, q * (IPG // 16) : (q + 1) * (IPG // 16)],
                    IPG,
                    IPG,
                    CHUNK,
                    queue_num=q,
                ).then_inc(s_g[q], 16)

        @blk.vector
        def _(v_eng):
            v_eng.wait_ge(s_c, 16)
            v_eng.tensor_tensor(
                mask.ap().rearrange("p (j v) -> p j v", j=NJ),
                iota_ap.rearrange("p (o v) -> p o v", o=1).broadcast_to([P, NJ, CHUNK]),
                labm_ap.rearrange("p (j o) -> p j o", o=1).broadcast_to([P, NJ, CHUNK]),
                Alu.is_equal,
            )
            v_eng.drain()
            for q in range(NQ):
                v_eng.wait_ge(s_g[q], 16)
                sl = slice(q * JPG * CHUNK, (q + 1) * JPG * CHUNK)
                v_eng.scalar_tensor_tensor(
                    prod.ap()[:, sl],
                    dst.ap()[:, sl],
                    -PEN,
                    mask.ap()[:, sl],
                    Alu.add,
                    Alu.mult,
                ).then_inc(s_sel, 1)

        @blk.scalar
        def _(a_eng):
            a_eng.add_instruction(
                mybir.InstLoadActFuncSet(
                    name=nc.get_next_instruction_name(),
                    ins=[],
                    outs=[],
                    act_func_set_id=NLE_TABLE_ID,
                )
            )
            for q in range(NQ):
                a_eng.wait_ge(s_sel, q + 1)
                sl = slice(q * JPG * CHUNK, (q + 1) * JPG * CHUNK)
                a_eng.activation(
                    u_big.ap()[:, sl], prod.ap()[:, sl], Act.Exp,
                    scale=-1.0, bias=npen_ap,
                )
            a_eng.drain()
            for q in range(NQ):
                sl = slice(q * JPG * CHUNK, (q + 1) * JPG * CHUNK)
                a_eng.activation(
                    ln_big.ap()[:, sl], u_big.ap()[:, sl], Act.Ln,
                    bias=1.0, accum_out=acc4.ap()[:, q : q + 1],
                )
            a_eng.drain()
            a_eng.activation(
                junk4.ap(), acc4.ap(), Act.Identity, accum_out=acc1.ap()
            ).then_inc(s_acc, 1)
            a_eng.wait_ge(s_mm, 1)
            a_eng.activation(outs.ap(), po.ap(), Act.Identity)
            a_eng.drain()
            a_eng.dma_start(out=out_d, in_=outs.ap()).then_inc(s_out, 16)
            a_eng.wait_ge(s_out, 16)

        @blk.tensor
        def _(t_eng):
            t_eng.wait_ge(s_acc, 1)
            t_eng.matmul(
                po.ap(), acc1.ap(), ones_ap, start=True, stop=True
            ).then_inc(s_mm, 1)

    nc.compile()
    return nc


def _get_nc():
    if "nc" not in _CACHE:
        _CACHE["nc"] = _build()
    return _CACHE["nc"]


def _shard_inputs(fs, labels):
    fs = np.ascontiguousarray(np.asarray(fs, dtype=np.float32))
    labels = np.asarray(labels).astype(np.int64)
    iota64 = np.tile(np.arange(CHUNK, dtype=np.float32), (P, 1))  # [128, 64]
    ones = np.ones((P, 1), dtype=np.float32)
    in_maps = []
    for c in range(N_CORES):
        fs_loc = fs[c * BL : (c + 1) * BL]
        lab = labels[c * BL : (c + 1) * BL]
        chunk = (lab >> 6).astype(np.int16)
        within = (lab & 63).astype(np.float32)
        gidx = (4 * np.arange(BL, dtype=np.int32) + chunk).astype(np.int16)
        # gather q covers positions i' = 0..1023 -> global i = q*1024 + i';
        # idx column = q*64 + i'//16, partition = i'%16 (replicated x8)
        idx_arr = np.empty((16, BL // 16), dtype=np.int16)
        for q in range(NQ):
            blkq = gidx[q * IPG : (q + 1) * IPG].reshape(IPG // 16, 16).T  # [16, 64]
            idx_arr[:, q * (IPG // 16) : (q + 1) * (IPG // 16)] = blkq
        idx_full = np.tile(idx_arr, (8, 1))  # [128, 256]
        # labm[p, q*8+jj] = within[q*1024 + jj*128 + p]
        labm = within.reshape(NQ, JPG, P).transpose(2, 0, 1).reshape(P, NJ)
        consts = np.concatenate([labm, iota64, ones, np.full((P, 1), -PEN, np.float32)], axis=1).astype(np.float32)
        in_maps.append(
            {
                "fs": fs_loc,
                "idx": np.ascontiguousarray(idx_full),
                "consts": np.ascontiguousarray(consts),
            }
        )
    return in_maps


def kernel(fs, labels, _trace=False, _trace_kwargs=None):
    from concourse.bass_utils import run_bass_kernel_spmd

    nc = _get_nc()
    in_maps = _shard_inputs(fs, labels)
    res = run_bass_kernel_spmd(
        nc,
        in_maps,
        core_ids=list(range(N_CORES)),
        trace=_trace,
        **(_trace_kwargs or {}),
    )
    total = np.float64(0.0)
    for c in range(N_CORES):
        total += np.float64(res.results[c]["out"][0, 0])
    loss = total / np.float64(B)
    if _trace:
        return np.float64(loss), res
    return np.asarray(loss, dtype=np.float64)


# revision 11
# speedup vs baseline: 2.5258x; 2.5258x over previous
"""HEX loss kernel for Trainium2 (8 NeuronCores, batch-parallel, raw Bass).

Math: the chain junction-tree potential is rank-1 per clique and each
interior fs[v] is split fs[v]/2 over its two cliques, so the joint
distribution factorizes into independent Bernoullis with
P(y_v=1) = sigmoid(fs[b,v]); hence
    loss = mean_b softplus(-fs[b, labels[b]])

Implementation (per core, 4096 rows): fs is host-cast to bf16 (2 MB)
and streamed via 4 direct SWDGE DMAs at ~300 GB/s (dma_gather was
measured 3-6x slower: ~10 ns/descriptor Q7 gen + ~50-100 GB/s random
256B reads). The label select runs as bf16 big-tile ops: DVE builds
one-hot masks with is_equal(iota256, labt) broadcast APs, multiplies,
and group-reduces to sel = fs[b, lab_b]; two of the four reduces run
on the otherwise-idle GpSimd(Pool) engine. softplus(-sel) =
Ln(1 + Exp(-sel)) runs on ACT (the gen3 natural_log_exp table set has
both Exp and Ln; the table load is issued manually at t=0 so it
overlaps the input DMAs), with accum_out producing [128,1] partials.
A PE matmul against a ones-vector reduces [128,1] -> [1,1] in PSUM so
the final output DMA is a single descriptor -- a [128,1] DRAM store
was measured at 7-10 us of 16-engine completion-sem stagger, vs ~1.8
us for the [1,1] store. Host sums 8 scalars / B.
"""

import numpy as np

B = 32768
V = 256
N_CORES = 8
BL = B // N_CORES          # 4096 rows per core
P = 128
NG = 4                     # stream groups
RPP = 8                    # rows per partition per group
GCOLS = RPP * V            # 2048 bf16 els per partition per group
NT = BL // P               # 32 sel columns total
NLE_TABLE_ID = 6           # natural_log_exp_and_others in gen3 act_info

_CACHE = {}


def _build():
    from contextlib import ExitStack

    import concourse.bass as bass  # noqa
    from concourse import bacc, mybir

    f32 = mybir.dt.float32
    bf16 = mybir.dt.bfloat16
    Alu = mybir.AluOpType
    Act = mybir.ActivationFunctionType

    nc = bacc.Bacc(
        "TRN2",
        target_bir_lowering=False,
        debug=False,
        enable_asserts=True,
        num_devices=N_CORES,
    )

    fs_d = nc.dram_tensor("fs", [BL, V], bf16, kind="ExternalInput").ap()
    cb_d = nc.dram_tensor("cb", [P, NT + V], bf16, kind="ExternalInput").ap()
    ones_d = nc.dram_tensor("ones", [P, 1], f32, kind="ExternalInput").ap()
    out_d = nc.dram_tensor("out", [1, 1], f32, kind="ExternalOutput").ap()

    # row = g*1024 + p*8 + j  ->  group tile [128, (j v)]
    fs_view = fs_d.rearrange("(g p j) v -> g p (j v)", g=NG, p=P, j=RPP)

    with ExitStack() as ctx:
        fs_t = [
            ctx.enter_context(nc.sbuf_tensor(f"fs_t{g}", [P, GCOLS], bf16))
            for g in range(NG)
        ]
        cb = ctx.enter_context(nc.sbuf_tensor("cb_sb", [P, NT + V], bf16))
        onesb = ctx.enter_context(nc.sbuf_tensor("ones_sb", [P, 1], f32))
        mask = ctx.enter_context(nc.sbuf_tensor("mask", [P, NG * GCOLS], bf16))
        prod = ctx.enter_context(nc.sbuf_tensor("prod", [P, NG * GCOLS], bf16))
        sel = ctx.enter_context(nc.sbuf_tensor("sel", [P, NT], f32))
        u32 = ctx.enter_context(nc.sbuf_tensor("u32", [P, NT], f32))
        l32 = ctx.enter_context(nc.sbuf_tensor("l32", [P, NT], f32))
        acc1 = ctx.enter_context(nc.sbuf_tensor("acc1", [P, 1], f32))
        outs = ctx.enter_context(nc.sbuf_tensor("outs", [1, 1], f32))
        po = ctx.enter_context(nc.psum_tensor("po", [1, 1], f32))

        s_c = ctx.enter_context(nc.semaphore("s_c"))
        s_o = ctx.enter_context(nc.semaphore("s_o"))
        s_f = [ctx.enter_context(nc.semaphore(f"s_f{g}")) for g in range(NG)]
        s_p = [ctx.enter_context(nc.semaphore(f"s_p{g}")) for g in range(NG)]
        s_sel = ctx.enter_context(nc.semaphore("s_sel"))
        s_acc = ctx.enter_context(nc.semaphore("s_acc"))
        s_mm = ctx.enter_context(nc.semaphore("s_mm"))
        s_out = ctx.enter_context(nc.semaphore("s_out"))

        labt_ap = cb.ap()[:, 0:NT]          # [128, 32] bf16 labels
        iota_ap = cb.ap()[:, NT : NT + V]   # [128, 256] bf16 iota

        blk = ctx.enter_context(nc.Block())

        @blk.sync
        def _(s_eng):
            s_eng.dma_start(out=cb.ap(), in_=cb_d).then_inc(s_c, 16)
            s_eng.dma_start(out=onesb.ap(), in_=ones_d).then_inc(s_o, 16)

        @blk.gpsimd
        def _(g_eng):
            for g in range(NG):
                g_eng.dma_start(out=fs_t[g].ap(), in_=fs_view[g]).then_inc(
                    s_f[g], 16
                )
            # Pool engine reduces for groups 1 and 3
            for g in (1, 3):
                g_eng.wait_ge(s_p[g], 1)
                g_eng.tensor_reduce(
                    sel.ap()[:, g * RPP : (g + 1) * RPP],
                    prod.ap()[:, g * GCOLS : (g + 1) * GCOLS].rearrange(
                        "p (j v) -> p j v", j=RPP
                    ),
                    axis=mybir.AxisListType.X,
                    op=Alu.add,
                ).then_inc(s_sel, 1)

        @blk.vector
        def _(v_eng):
            v_eng.wait_ge(s_c, 16)
            for g in range(NG):
                v_eng.tensor_tensor(
                    mask.ap()[:, g * GCOLS : (g + 1) * GCOLS].rearrange(
                        "p (j v) -> p j v", j=RPP
                    ),
                    iota_ap.rearrange("p (o v) -> p o v", o=1).broadcast_to(
                        [P, RPP, V]
                    ),
                    labt_ap[:, g * RPP : (g + 1) * RPP]
                    .rearrange("p (j o) -> p j o", o=1)
                    .broadcast_to([P, RPP, V]),
                    Alu.is_equal,
                )
            v_eng.drain()
            for g in range(NG):
                v_eng.wait_ge(s_f[g], 16)
                sl = slice(g * GCOLS, (g + 1) * GCOLS)
                v_eng.tensor_mul(
                    prod.ap()[:, sl], mask.ap()[:, sl], fs_t[g].ap()
                ).then_inc(s_p[g], 1)
            v_eng.drain()
            # DVE reduces for groups 0 and 2
            for g in (0, 2):
                v_eng.tensor_reduce(
                    sel.ap()[:, g * RPP : (g + 1) * RPP],
                    prod.ap()[:, g * GCOLS : (g + 1) * GCOLS].rearrange(
                        "p (j v) -> p j v", j=RPP
                    ),
                    axis=mybir.AxisListType.X,
                    op=Alu.add,
                ).then_inc(s_sel, 1)

        @blk.scalar
        def _(a_eng):
            a_eng.add_instruction(
                mybir.InstLoadActFuncSet(
                    name=nc.get_next_instruction_name(),
                    ins=[],
                    outs=[],
                    act_func_set_id=NLE_TABLE_ID,
                )
            )
            a_eng.wait_ge(s_sel, NG)
            a_eng.activation(u32.ap(), sel.ap(), Act.Exp, scale=-1.0)
            a_eng.drain()
            a_eng.activation(
                l32.ap(), u32.ap(), Act.Ln, bias=1.0, accum_out=acc1.ap()
            ).then_inc(s_acc, 1)
            a_eng.wait_ge(s_mm, 1)
            a_eng.activation(outs.ap(), po.ap(), Act.Identity)
            a_eng.drain()
            a_eng.dma_start(out=out_d, in_=outs.ap()).then_inc(s_out, 16)
            a_eng.wait_ge(s_out, 16)

        @blk.tensor
        def _(t_eng):
            t_eng.wait_ge(s_acc, 1)
            t_eng.wait_ge(s_o, 16)
            t_eng.matmul(
                po.ap(), acc1.ap(), onesb.ap(), start=True, stop=True
            ).then_inc(s_mm, 1)

    nc.compile()
    return nc


def _get_nc():
    if "nc" not in _CACHE:
        _CACHE["nc"] = _build()
    return _CACHE["nc"]


def _shard_inputs(fs, labels):
    import ml_dtypes

    fs = np.asarray(fs, dtype=np.float32)
    labels = np.asarray(labels).astype(np.int64)
    iota256 = np.tile(np.arange(V, dtype=np.float32), (P, 1))  # [128, 256]
    ones = np.ones((P, 1), dtype=np.float32)
    in_maps = []
    for c in range(N_CORES):
        fs_loc = np.ascontiguousarray(fs[c * BL : (c + 1) * BL]).astype(
            ml_dtypes.bfloat16
        )
        lab = labels[c * BL : (c + 1) * BL]
        # labt[p, g*8+j] = lab[g*1024 + p*8 + j]
        labt = (
            lab.reshape(NG, P, RPP).transpose(1, 0, 2).reshape(P, NT)
        ).astype(np.float32)
        cb = np.concatenate([labt, iota256], axis=1).astype(ml_dtypes.bfloat16)
        in_maps.append(
            {
                "fs": fs_loc,
                "cb": np.ascontiguousarray(cb),
                "ones": ones,
            }
        )
    return in_maps


def kernel(fs, labels, _trace=False, _trace_kwargs=None):
    from concourse.bass_utils import run_bass_kernel_spmd

    nc = _get_nc()
    in_maps = _shard_inputs(fs, labels)
    res = run_bass_kernel_spmd(
        nc,
        in_maps,
        core_ids=list(range(N_CORES)),
        trace=_trace,
        **(_trace_kwargs or {}),
    )
    total = np.float64(0.0)
    for c in range(N_CORES):
        total += np.float64(res.results[c]["out"][0, 0])
    loss = total / np.float64(B)
    if _trace:
        return np.float64(loss), res
    return np.asarray(loss, dtype=np.float64)
